# revision 1
# baseline (speedup 1.0000x reference)
"""Trainium2 Bass kernel for BiGNNLayer (COO SpMM + dense mix).

Computes, for L given in COO form (lap_rows=dest, lap_cols=src, lap_vals):
    x   = segment_sum(lap_vals * features[lap_cols], lap_rows)   # L @ F
    out = (features + x) @ W1 + b1 + (x * features) @ W2 + b2

Sharding: dest nodes are striped across the 8 cores by global degree rank
(rank r -> core r%8, position r//8), so every core gets exactly 12500 dests
with near-identical degree profiles; edges are partitioned by dest core on
the host; the feature table is replicated into every core's HBM so no
device collectives are needed.

Per-core SPMD kernel: positions are degree-sorted 128-row tiles; each
tile's edges form a ragged slot matrix [128 x K_t] where K_t is the global
rank-1024t degree (identical across cores, keeping the program SPMD-
uniform and the total slot-column count within ~0.2% of the edges/128
floor).  Slot column k of tile t is fetched with one vector-offset DMA
(128 dynamic int32 row offsets, one per partition - the only indirect-DMA
form this hardware executes).  Slots are scaled by vals and reduced over k
with a strided tensor_reduce.  The dense part (PE transpose of x tiles +
two accumulating matmuls with W1/W2 stationary) is emitted with live
buffers alongside phase A so the scheduler hides it under the gather
stream; the host un-permutes and un-transposes the output.
"""

import sys

sys.path.insert(0, "/opt/trn_rl_repo")

import numpy as np

import concourse.bacc as bacc
import concourse.tile as tile
from concourse import bass, mybir
from concourse.bass import IndirectOffsetOnAxis
from concourse.bass_utils import run_bass_kernel_spmd

# ---------------- problem constants (hardcoded per the contract) -----------
N_NODES = 100000
N_EDGES = 3200000
D = 64
CORES = 8
ND = N_NODES // CORES          # 12500 dest rows per core
T_ROWS = (ND + 127) // 128     # 98 row tiles (12544 padded rows)
NDP = T_ROWS * 128

FP32 = mybir.dt.float32
INT32 = mybir.dt.int32


# ---------------------------- host prep ------------------------------------
def _prep(lap_rows, lap_cols, lap_vals, features, W1, b1, W2, b2):
    lap_rows = np.ascontiguousarray(lap_rows)
    lap_cols = np.ascontiguousarray(lap_cols)
    lap_vals = np.ascontiguousarray(lap_vals)
    features = np.ascontiguousarray(features, dtype=np.float32)

    # global degree-rank striping: rank r -> core r%8, position r//8
    deg = np.bincount(lap_rows, minlength=N_NODES)
    gorder = np.argsort(-deg, kind="stable")
    grank = np.empty(N_NODES, np.int64)
    grank[gorder] = np.arange(N_NODES)
    core_of = (grank % CORES).astype(np.int64)
    pos_of = grank // CORES                      # 0..ND-1 within core

    # K_t identical across cores: tile t's max degree = degree at rank 1024t
    deg_sorted = deg[gorder]
    kt = np.maximum(deg_sorted[np.arange(T_ROWS) * 128 * CORES], 1)
    offs = np.zeros(T_ROWS + 1, np.int64)
    np.cumsum(kt, out=offs[1:])
    ksum = int(offs[-1])

    ecore = core_of[lap_rows]
    order = np.argsort(ecore, kind="stable")
    bounds = np.searchsorted(ecore[order], np.arange(CORES + 1))
    pos_s = pos_of[lap_rows[order]]
    cols_s = lap_cols[order]
    vals_s = lap_vals[order]

    bias = (np.asarray(b1, np.float32) + np.asarray(b2, np.float32)).reshape(D, 1)
    W1 = np.ascontiguousarray(W1, np.float32)
    W2 = np.ascontiguousarray(W2, np.float32)
    ident = np.eye(128, dtype=np.float32)

    in_maps = []
    perms = []
    for c in range(CORES):
        lo, hi = bounds[c], bounds[c + 1]
        pos = pos_s[lo:hi]                       # dest position within core
        cols = cols_s[lo:hi]
        vals = vals_s[lo:hi]
        o2 = np.argsort(pos, kind="stable")
        p2 = pos[o2]
        cdeg = np.bincount(p2, minlength=ND)
        starts = np.zeros(ND, np.int64)
        np.cumsum(cdeg[:-1], out=starts[1:])
        rank = np.arange(p2.shape[0]) - starts[p2]

        t = p2 // 128
        p = p2 % 128
        if np.any(rank >= kt[t]):
            raise AssertionError("slot overflow: degree exceeds tile budget")
        gidx = np.zeros((128, ksum), np.int32)
        gval = np.zeros((128, ksum), np.float32)
        col_idx = offs[t] + rank
        gidx[p, col_idx] = cols[o2]
        gval[p, col_idx] = vals[o2]

        perm = gorder[c::CORES]                  # position i -> dest id
        fT = np.zeros((D, NDP), np.float32)
        fT[:, :ND] = features[perm].T
        perms.append(perm)

        in_maps.append(
            {
                "feat": features,
                "gidx": gidx,
                "gval": gval,
                "fT": fT,
                "W1": W1,
                "W2": W2,
                "bias": bias,
                "ident": ident,
            }
        )
    return in_maps, perms, kt.tolist(), offs.tolist(), ksum


# --------------------------- device kernel ---------------------------------
def build_kernel(kt, offs, ksum):
    nc = bacc.Bacc("TRN2", target_bir_lowering=False, debug=False)

    feat = nc.dram_tensor("feat", [N_NODES, D], FP32, kind="ExternalInput")
    gidx = nc.dram_tensor("gidx", [128, ksum], INT32, kind="ExternalInput")
    gval = nc.dram_tensor("gval", [128, ksum], FP32, kind="ExternalInput")
    fT = nc.dram_tensor("fT", [D, NDP], FP32, kind="ExternalInput")
    W1 = nc.dram_tensor("W1", [D, D], FP32, kind="ExternalInput")
    W2 = nc.dram_tensor("W2", [D, D], FP32, kind="ExternalInput")
    bias = nc.dram_tensor("bias", [D, 1], FP32, kind="ExternalInput")
    ident = nc.dram_tensor("ident", [128, 128], FP32, kind="ExternalInput")

    outT = nc.dram_tensor("outT", [D, NDP], FP32, kind="ExternalOutput")

    kmax = max(kt)

    with tile.TileContext(nc) as tc:
        with (
            tc.tile_pool(name="acc", bufs=1) as apool,
            tc.tile_pool(name="dense", bufs=1) as dpool,
            tc.tile_pool(name="gbuf", bufs=2) as gpool,
            tc.tile_pool(name="meta", bufs=1) as mpool,
            tc.tile_pool(name="dwork", bufs=3) as wpool,
            tc.tile_pool(name="psum", bufs=4, space="PSUM") as pspool,
        ):
            x_acc = apool.tile([128, T_ROWS * D], FP32)

            # split the offset load so tile 0's gathers start immediately
            k0 = kt[0]
            idx_sb = mpool.tile([128, ksum], INT32)
            nc.sync.dma_start(out=idx_sb[:, :k0], in_=gidx[:, :k0])
            nc.sync.dma_start(out=idx_sb[:, k0:], in_=gidx[:, k0:])
            val_sb = mpool.tile([128, ksum], FP32)
            nc.sync.dma_start(out=val_sb[:], in_=gval[:])

            fT_sb = dpool.tile([D, NDP], FP32)
            nc.sync.dma_start(out=fT_sb[:], in_=fT[:])
            w1_sb = dpool.tile([D, D], FP32)
            nc.sync.dma_start(out=w1_sb[:], in_=W1[:])
            w2_sb = dpool.tile([D, D], FP32)
            nc.sync.dma_start(out=w2_sb[:], in_=W2[:])
            bias_sb = dpool.tile([D, 1], FP32)
            nc.sync.dma_start(out=bias_sb[:], in_=bias[:])
            id_sb = dpool.tile([128, 128], FP32)
            nc.sync.dma_start(out=id_sb[:], in_=ident[:])
            outT_sb = dpool.tile([D, NDP], FP32)

            def dense_tile(i):
                xT_ps = pspool.tile([D, 128], FP32, tag="xT")
                nc.tensor.transpose(
                    out=xT_ps[:],
                    in_=x_acc[:, i * D : (i + 1) * D],
                    identity=id_sb[:],
                )
                fslice = fT_sb[:, i * 128 : (i + 1) * 128]
                a_t = wpool.tile([D, 128], FP32, tag="A")
                nc.vector.tensor_tensor(
                    out=a_t[:], in0=fslice, in1=xT_ps[:], op=mybir.AluOpType.add
                )
                b_t = wpool.tile([D, 128], FP32, tag="B")
                nc.vector.tensor_tensor(
                    out=b_t[:], in0=fslice, in1=xT_ps[:], op=mybir.AluOpType.mult
                )
                o_ps = pspool.tile([D, 128], FP32, tag="o")
                nc.tensor.matmul(
                    o_ps[:], lhsT=w1_sb[:], rhs=a_t[:], start=True, stop=False
                )
                nc.tensor.matmul(
                    o_ps[:], lhsT=w2_sb[:], rhs=b_t[:], start=False, stop=True
                )
                nc.vector.tensor_scalar_add(
                    outT_sb[:, i * 128 : (i + 1) * 128],
                    o_ps[:],
                    bias_sb[:],
                )
                # stream this tile's output out immediately (hides the
                # final store under the remaining gather stream)
                nc.sync.dma_start(
                    out=outT[:, i * 128 : (i + 1) * 128],
                    in_=outT_sb[:, i * 128 : (i + 1) * 128],
                )

            for t in range(T_ROWS):
                K = kt[t]
                off = offs[t]
                G = gpool.tile([128, kmax * D], FP32, tag="G")
                for k in range(K):
                    nc.gpsimd.indirect_dma_start(
                        out=G[:, k * D : (k + 1) * D],
                        out_offset=None,
                        in_=feat[:],
                        in_offset=IndirectOffsetOnAxis(
                            ap=idx_sb[:, off + k : off + k + 1], axis=0
                        ),
                    )
                G3 = G[:].rearrange("p (k f) -> p k f", f=D)[:, :K, :]
                nc.vector.tensor_tensor(
                    out=G3,
                    in0=G3,
                    in1=val_sb[:, off : off + K, None].to_broadcast([128, K, D]),
                    op=mybir.AluOpType.mult,
                )
                gview = G[:].rearrange("p (k f) -> p f k", k=kmax, f=D)[:, :, :K]
                nc.vector.tensor_reduce(
                    out=x_acc[:, t * D : (t + 1) * D],
                    in_=gview,
                    axis=mybir.AxisListType.X,
                    op=mybir.AluOpType.add,
                )
                dense_tile(t)

    nc.compile()
    return nc


# ------------------------------ entry point --------------------------------
def kernel(lap_rows, lap_cols, lap_vals, features, W1, b1, W2, b2):
    in_maps, perms, kt, offs, ksum = _prep(
        lap_rows, lap_cols, lap_vals, features, W1, b1, W2, b2
    )
    nc = build_kernel(kt, offs, ksum)
    res = run_bass_kernel_spmd(nc, in_maps, core_ids=list(range(CORES)))
    out = np.empty((N_NODES, D), np.float32)
    for c in range(CORES):
        out[perms[c]] = res.results[c]["outT"][:, :ND].T
    return out


if __name__ == "__main__":
    import reference

    inp = reference.setup_inputs()
    inp = {k: np.asarray(v) for k, v in inp.items()}
    got = kernel(**inp)
    print("kernel ran, output shape", got.shape)



# revision 9
# speedup vs baseline: 4.5224x; 4.5224x over previous
"""Trainium2 Bass kernel for BiGNNLayer (COO SpMM + dense mix).

Computes, for L given in COO form (lap_rows=dest, lap_cols=src, lap_vals):
    x   = segment_sum(lap_vals * features[lap_cols], lap_rows)   # L @ F
    out = (features + x) @ W1 + b1 + (x * features) @ W2 + b2

Strategy (dest-sharded, feature table replicated, no collectives):
  * The 3.2M-edge gather is done with batched SWDGE `dma_gather`
    (InstDMAGatherAnt) instead of per-column indirect DMAs - one instruction
    fetches thousands of 256B rows, amortizing the ~1us Pool-engine
    descriptor-generation overhead that dominated the per-column approach.
  * dma_gather indices are int16 (<=32767) but the table has 100k rows; the
    table is viewed as [25002, 256] (4 rows per record) and each gather
    instruction uses one of 4 column slices (64*r .. 64*r+64), reaching rows
    4*j+r. A source can only be fetched by class r = (row % 4), so the host
    RELABELS nodes (permutes the table) choosing each node's class with a
    greedy balancer that equalizes per-dest class counts.
  * Dest nodes are grouped into 128-row tiles by (max class count, argmax,
    B-vector) so all 128 dests of a tile (x8 cores) have near-identical
    class-count vectors; per-(tile,class) slot-column budgets K then carry
    only ~16% padding.
  * Per supertile (group of tiles, ~256 slot columns): 4 gathers (one per
    class) fill a [128, C, 64] slot matrix; one bulk multiply scales by edge
    values (padded slots have val=0); per-(tile,class) strided tensor_reduce
    plus a 4-way combine produce x for the tile's 128 dests.
  * Dense tail per tile: PE transpose of x, fT add/mult, two accumulating
    matmuls with W1/W2 stationary, bias add, streamed output store.
"""

import sys

sys.path.insert(0, "/opt/trn_rl_repo")

import numpy as np

import concourse.bacc as bacc
import concourse.tile as tile
from concourse import bass, library_config, mybir
from concourse.bass_utils import run_bass_kernel_spmd

# ---------------- problem constants (hardcoded per the contract) -----------
N_NODES = 100000
N_EDGES = 3200000
D = 64
CORES = 8
ND = N_NODES // CORES          # 12500 dest rows per core
T_ROWS = (ND + 127) // 128     # 98 row tiles
NDP = T_ROWS * 128
NCLS = 4
TAB_ROWS = N_NODES // NCLS + 2  # 25002 records of 4 rows
CMAX = 48                       # max slot columns per supertile

FP32 = mybir.dt.float32
INT16 = mybir.dt.int16

_rows16 = np.arange(16)


def _balance_classes(lap_rows, lap_cols):
    """Greedy per-source class assignment equalizing per-dest class counts."""
    order = np.argsort(lap_cols, kind="stable")
    dst_by_src = lap_rows[order]
    starts = np.searchsorted(lap_cols[order], np.arange(N_NODES + 1))
    B = np.zeros((N_NODES, NCLS), np.int32)
    cls = np.empty(N_NODES, np.int8)
    cap = np.full(NCLS, N_NODES // NCLS, np.int64)
    odeg = np.diff(starts)
    src_order = np.argsort(-odeg, kind="stable")
    for sid in src_order:
        ds = dst_by_src[starts[sid] : starts[sid + 1]]
        if len(ds) == 0:
            j = int(np.argmax(cap))
            cls[sid] = j
            cap[j] -= 1
            continue
        Bd = B[ds]
        rowmean = Bd.mean(1, keepdims=True)
        sc = np.maximum(Bd + 1.0 - rowmean, 0.0).sum(0) + 0.02 * Bd.sum(0)
        sc = sc + np.where(cap > 0, 0.0, 1e18)
        j = int(np.argmin(sc))
        cls[sid] = j
        cap[j] -= 1
        np.add.at(B, (ds, j), 1)
    return cls, B


# ---------------------------- host prep ------------------------------------
def _prep(lap_rows, lap_cols, lap_vals, features, W1, b1, W2, b2):
    lap_rows = np.ascontiguousarray(np.asarray(lap_rows))
    lap_cols = np.ascontiguousarray(np.asarray(lap_cols))
    lap_vals = np.ascontiguousarray(np.asarray(lap_vals), dtype=np.float32)
    features = np.ascontiguousarray(np.asarray(features), dtype=np.float32)

    cls, B = _balance_classes(lap_rows, lap_cols)

    # source relabeling: class r gets table rows r, r+4, ...
    newid = np.empty(N_NODES, np.int64)
    for r in range(NCLS):
        m = np.where(cls == r)[0]
        newid[m] = r + 4 * np.arange(len(m))
    featP = np.zeros((TAB_ROWS * 4, D), np.float32)
    featP[newid] = features
    tab = featP.reshape(TAB_ROWS, 4 * D)

    # dest striping: group by (max class count desc, argmax, B-vector)
    mx = B.max(1)
    am = B.argmax(1)
    sorder = np.lexsort((B[:, 2], B[:, 1], B[:, 0], am, -mx))
    grank = np.empty(N_NODES, np.int64)
    grank[sorder] = np.arange(N_NODES)
    # rank -> core = rank%8, pos = rank//8, tile = pos//128, part = pos%128

    # global budgets K[t, r] = max over the tile's 1024 dests (SPMD-uniform)
    K = np.zeros((T_ROWS, NCLS), np.int64)
    for t in range(T_ROWS):
        lo, hi = t * 1024, min((t + 1) * 1024, N_NODES)
        K[t] = B[sorder[lo:hi]].max(0)
    K = np.maximum(K, 1)

    # supertile grouping and global column layout
    tile_cols = K.sum(1)
    supers = []  # list of (t_lo, t_hi)
    t0 = 0
    acc = 0
    for t in range(T_ROWS):
        if acc and acc + tile_cols[t] > CMAX:
            supers.append((t0, t))
            t0, acc = t, 0
        acc += tile_cols[t]
    supers.append((t0, T_ROWS))

    colbase = np.zeros((T_ROWS, NCLS), np.int64)
    sinfo = []  # per super: (col_lo, col_hi, [(r, col_lo_r, col_hi_r)])
    col = 0
    for (ta, tb) in supers:
        s_lo = col
        blocks = []
        for r in range(NCLS):
            r_lo = col
            for t in range(ta, tb):
                colbase[t, r] = col
                col += K[t, r]
            blocks.append((r, r_lo, col))
        sinfo.append((s_lo, col, blocks))
    totcols = col

    # per-edge placement
    e_rank = grank[lap_rows]
    e_core = e_rank % CORES
    e_pos = e_rank // CORES
    e_t = e_pos // 128
    e_p = e_pos % 128
    e_cls = (newid[lap_cols] % 4).astype(np.int64)
    e_i16 = (newid[lap_cols] // 4).astype(np.int64)

    bias = (np.asarray(b1, np.float32) + np.asarray(b2, np.float32)).reshape(D, 1)
    W1 = np.ascontiguousarray(np.asarray(W1), np.float32)
    W2 = np.ascontiguousarray(np.asarray(W2), np.float32)
    ident = np.eye(128, dtype=np.float32)

    in_maps = []
    perms = []
    for c in range(CORES):
        sel = np.where(e_core == c)[0]
        t_ = e_t[sel]
        p_ = e_p[sel]
        r_ = e_cls[sel]
        i16_ = e_i16[sel]
        v_ = lap_vals[sel]
        # rank within (t, r, p) groups
        key = (t_ * NCLS + r_) * 128 + p_
        o2 = np.argsort(key, kind="stable")
        ks = key[o2]
        grp_starts = np.searchsorted(ks, np.arange(T_ROWS * NCLS * 128))
        rank_in = np.arange(len(ks)) - grp_starts[ks]
        colidx = colbase[t_[o2], r_[o2]] + rank_in
        if np.any(rank_in >= K[t_[o2], r_[o2]]):
            raise AssertionError("slot overflow")
        gidx = np.zeros((128, totcols), np.int16)
        gval = np.zeros((128, totcols), np.float32)
        gidx[p_[o2], colidx] = i16_[o2]
        gval[p_[o2], colidx] = v_[o2]

        # idx buffer in dma_gather layout: flat i = (col-a)*128 + p within
        # each per-(super, class) instruction block; [16, 8C] replicated x8
        idxbuf = np.empty((128, 8 * totcols), np.int16)
        for (s_lo, s_hi, blocks) in sinfo:
            for (r, r_lo, r_hi) in blocks:
                C = r_hi - r_lo
                if C == 0:
                    continue
                arr = gidx[:, r_lo:r_hi].T.ravel()  # i = c*128+p
                r16 = arr.reshape(C * 8, 16).T      # [16, 8C]
                idxbuf[:, 8 * r_lo : 8 * r_hi] = np.tile(r16, (8, 1))

        perm = sorder[c::CORES]  # position i -> dest id
        fT = np.zeros((D, NDP), np.float32)
        fT[:, :ND] = features[perm].T
        perms.append(perm)

        in_maps.append(
            {
                "tab": tab,
                "idx": idxbuf,
                "val": gval,
                "fT": fT,
                "W1": W1,
                "W2": W2,
                "bias": bias,
                "ident": ident,
            }
        )
    meta = {
        "K": K.tolist(),
        "supers": supers,
        "sinfo": sinfo,
        "colbase": colbase.tolist(),
        "totcols": totcols,
    }
    return in_maps, perms, meta


# --------------------------- device kernel ---------------------------------
def build_kernel(meta, gbufs=4):
    K = meta["K"]
    sinfo = meta["sinfo"]
    supers = meta["supers"]
    colbase = meta["colbase"]
    totcols = meta["totcols"]

    nc = bacc.Bacc("TRN2", target_bir_lowering=False, debug=False)

    tab = nc.dram_tensor("tab", [TAB_ROWS, 4 * D], FP32, kind="ExternalInput")
    idx_t = nc.dram_tensor("idx", [128, 8 * totcols], INT16, kind="ExternalInput")
    val_t = nc.dram_tensor("val", [128, totcols], FP32, kind="ExternalInput")
    fT = nc.dram_tensor("fT", [D, NDP], FP32, kind="ExternalInput")
    W1 = nc.dram_tensor("W1", [D, D], FP32, kind="ExternalInput")
    W2 = nc.dram_tensor("W2", [D, D], FP32, kind="ExternalInput")
    bias = nc.dram_tensor("bias", [D, 1], FP32, kind="ExternalInput")
    ident = nc.dram_tensor("ident", [128, 128], FP32, kind="ExternalInput")
    outT = nc.dram_tensor("outT", [D, NDP], FP32, kind="ExternalOutput")

    with tile.TileContext(nc) as tc:
        nc.gpsimd.load_library(library_config.mlp)
        with (
            tc.tile_pool(name="dense", bufs=1) as dpool,
            tc.tile_pool(name="gbuf", bufs=gbufs) as gpool,
            tc.tile_pool(name="ibuf", bufs=gbufs) as ipool,
            tc.tile_pool(name="vbuf", bufs=gbufs) as vpool,
            tc.tile_pool(name="xbuf", bufs=3) as xpool,
            tc.tile_pool(name="fbuf", bufs=3) as fpool,
            tc.tile_pool(name="obuf", bufs=3) as opool,
            tc.tile_pool(name="work", bufs=3) as wpool,
            tc.tile_pool(name="psum", bufs=4, space="PSUM") as pspool,
        ):
            w1_sb = dpool.tile([D, D], FP32)
            nc.sync.dma_start(out=w1_sb[:], in_=W1[:])
            w2_sb = dpool.tile([D, D], FP32)
            nc.sync.dma_start(out=w2_sb[:], in_=W2[:])
            bias_sb = dpool.tile([D, 1], FP32)
            nc.sync.dma_start(out=bias_sb[:], in_=bias[:])
            id_sb = dpool.tile([128, 128], FP32)
            nc.sync.dma_start(out=id_sb[:], in_=ident[:])

            def dense_tile(t, x_t):
                fT_t = fpool.tile([D, 128], FP32, tag="f")
                nc.sync.dma_start(out=fT_t[:], in_=fT[:, t * 128 : (t + 1) * 128])
                xT_ps = pspool.tile([D, 128], FP32, tag="xT")
                nc.tensor.transpose(out=xT_ps[:], in_=x_t[:], identity=id_sb[:])
                a_t = wpool.tile([D, 128], FP32, tag="A")
                nc.vector.tensor_tensor(
                    out=a_t[:], in0=fT_t[:], in1=xT_ps[:], op=mybir.AluOpType.add
                )
                b_t = wpool.tile([D, 128], FP32, tag="B")
                nc.vector.tensor_tensor(
                    out=b_t[:], in0=fT_t[:], in1=xT_ps[:], op=mybir.AluOpType.mult
                )
                o_ps = pspool.tile([D, 128], FP32, tag="o")
                nc.tensor.matmul(
                    o_ps[:], lhsT=w1_sb[:], rhs=a_t[:], start=True, stop=False
                )
                nc.tensor.matmul(
                    o_ps[:], lhsT=w2_sb[:], rhs=b_t[:], start=False, stop=True
                )
                ot = opool.tile([D, 128], FP32, tag="ot")
                nc.scalar.activation(
                    out=ot[:],
                    in_=o_ps[:],
                    func=mybir.ActivationFunctionType.Identity,
                    bias=bias_sb[:],
                )
                nc.sync.dma_start(out=outT[:, t * 128 : (t + 1) * 128], in_=ot[:])

            for si, (ta, tb) in enumerate(supers):
                s_lo, s_hi, blocks = sinfo[si]
                C_s = s_hi - s_lo
                idx_sb = ipool.tile([128, 8 * C_s], INT16, tag="idx")
                nc.sync.dma_start(
                    out=idx_sb[:], in_=idx_t[:, 8 * s_lo : 8 * s_hi]
                )
                val_sb = vpool.tile([128, C_s], FP32, tag="val")
                nc.sync.dma_start(out=val_sb[:], in_=val_t[:, s_lo:s_hi])
                G = gpool.tile([128, C_s * D], FP32, tag="G")
                for (r, r_lo, r_hi) in blocks:
                    C_r = r_hi - r_lo
                    if C_r == 0:
                        continue
                    nc.gpsimd.dma_gather(
                        out_ap=G[:, (r_lo - s_lo) * D : (r_hi - s_lo) * D].rearrange(
                            "p (c f) -> p c f", f=D
                        ),
                        in_ap=tab[:][:, r * D : (r + 1) * D],
                        idxs_ap=idx_sb[:, 8 * (r_lo - s_lo) : 8 * (r_hi - s_lo)],
                        num_idxs=128 * C_r,
                        num_idxs_reg=128 * C_r,
                        elem_size=D,
                        elem_step=4 * D,
                        single_packet=False,
                    )
                G3 = G[:].rearrange("p (c f) -> p c f", f=D)
                nc.vector.tensor_tensor(
                    out=G3,
                    in0=G3,
                    in1=val_sb[:, :, None].to_broadcast([128, C_s, D]),
                    op=mybir.AluOpType.mult,
                )
                for t in range(ta, tb):
                    xpart = xpool.tile([128, NCLS * D], FP32, tag="xp")
                    for r in range(NCLS):
                        a = colbase[t][r] - s_lo
                        kk = K[t][r]
                        gv = G[:].rearrange("p (c f) -> p f c", f=D)[
                            :, :, a : a + kk
                        ]
                        nc.vector.tensor_reduce(
                            out=xpart[:, r * D : (r + 1) * D],
                            in_=gv,
                            axis=mybir.AxisListType.X,
                            op=mybir.AluOpType.add,
                        )
                    x_t = xpool.tile([128, D], FP32, tag="x")
                    nc.vector.tensor_reduce(
                        out=x_t[:],
                        in_=xpart[:].rearrange("p (c f) -> p f c", f=D),
                        axis=mybir.AxisListType.X,
                        op=mybir.AluOpType.add,
                    )
                    dense_tile(t, x_t)

    nc.compile()
    return nc


# ------------------------------ entry point --------------------------------
def kernel(lap_rows, lap_cols, lap_vals, features, W1, b1, W2, b2):
    in_maps, perms, meta = _prep(
        lap_rows, lap_cols, lap_vals, features, W1, b1, W2, b2
    )
    nc = build_kernel(meta)
    res = run_bass_kernel_spmd(nc, in_maps, core_ids=list(range(CORES)))
    out = np.empty((N_NODES, D), np.float32)
    for c in range(CORES):
        out[perms[c]] = res.results[c]["outT"][:, :ND].T
    return out


if __name__ == "__main__":
    # quick self-check on a numpy-generated graph (same distribution)
    rng = np.random.default_rng(0)
    inp = {
        "lap_rows": rng.integers(0, N_NODES, N_EDGES).astype(np.int32),
        "lap_cols": rng.integers(0, N_NODES, N_EDGES).astype(np.int32),
        "lap_vals": (rng.random(N_EDGES, np.float32) * 0.1),
        "features": rng.standard_normal((N_NODES, D)).astype(np.float32),
        "W1": (rng.standard_normal((D, D)) / np.sqrt(D)).astype(np.float32),
        "b1": np.zeros(D, np.float32),
        "W2": (rng.standard_normal((D, D)) / np.sqrt(D)).astype(np.float32),
        "b2": np.zeros(D, np.float32),
    }
    got = kernel(**inp)
    msgs = inp["lap_vals"][:, None] * inp["features"][inp["lap_cols"]]
    x = np.zeros_like(inp["features"])
    np.add.at(x, inp["lap_rows"], msgs)
    want = (
        (inp["features"] + x) @ inp["W1"]
        + inp["b1"]
        + (x * inp["features"]) @ inp["W2"]
        + inp["b2"]
    )
    err = np.abs(got - want)
    rel = err.max() / np.abs(want).max()
    print(f"max abs err {err.max():.3e}  rel {rel:.3e}")


# revision 15
# speedup vs baseline: 4.6393x; 1.0259x over previous
"""Trainium2 Bass kernel for BiGNNLayer (COO SpMM + dense mix).

Computes, for L given in COO form (lap_rows=dest, lap_cols=src, lap_vals):
    x   = segment_sum(lap_vals * features[lap_cols], lap_rows)   # L @ F
    out = (features + x) @ W1 + b1 + (x * features) @ W2 + b2

Strategy (dest-sharded, feature table replicated, no collectives):
  * The 3.2M-edge gather is done with batched SWDGE `dma_gather`
    (InstDMAGatherAnt) instead of per-column indirect DMAs - one instruction
    fetches thousands of 256B rows, amortizing the ~1us Pool-engine
    descriptor-generation overhead that dominated the per-column approach.
  * dma_gather indices are int16 (<=32767) but the table has 100k rows; the
    table is viewed as [25002, 256] (4 rows per record) and each gather
    instruction uses one of 4 column slices (64*r .. 64*r+64), reaching rows
    4*j+r. A source can only be fetched by class r = (row % 4), so the host
    RELABELS nodes (permutes the table) choosing each node's class with a
    greedy balancer that equalizes per-dest class counts.
  * Dest nodes are grouped into 128-row tiles by (max class count, argmax,
    B-vector) so all 128 dests of a tile (x8 cores) have near-identical
    class-count vectors; per-(tile,class) slot-column budgets K then carry
    only ~16% padding.
  * Per supertile (group of tiles, ~256 slot columns): 4 gathers (one per
    class) fill a [128, C, 64] slot matrix; one bulk multiply scales by edge
    values (padded slots have val=0); per-(tile,class) strided tensor_reduce
    plus a 4-way combine produce x for the tile's 128 dests.
  * Dense tail per tile: PE transpose of x, fT add/mult, two accumulating
    matmuls with W1/W2 stationary, bias add, streamed output store.
"""

import sys

sys.path.insert(0, "/opt/trn_rl_repo")

import numpy as np

import concourse.bacc as bacc
import concourse.tile as tile
from concourse import bass, library_config, mybir
from concourse.bass_utils import run_bass_kernel_spmd

# ---------------- problem constants (hardcoded per the contract) -----------
N_NODES = 100000
N_EDGES = 3200000
D = 64
CORES = 8
ND = N_NODES // CORES          # 12500 dest rows per core
T_ROWS = (ND + 127) // 128     # 98 row tiles
NDP = T_ROWS * 128
NCLS = 4
TAB_ROWS = N_NODES // NCLS + 2  # 25002 records of 4 rows
CMAX = 40                       # max slot columns per supertile

FP32 = mybir.dt.float32
INT16 = mybir.dt.int16

_rows16 = np.arange(16)


def _balance_classes(lap_rows, lap_cols):
    """Greedy per-source class assignment equalizing per-dest class counts."""
    order = np.argsort(lap_cols, kind="stable")
    dst_by_src = lap_rows[order]
    starts = np.searchsorted(lap_cols[order], np.arange(N_NODES + 1))
    B = np.zeros((N_NODES, NCLS), np.int32)
    cls = np.empty(N_NODES, np.int8)
    cap = np.full(NCLS, N_NODES // NCLS, np.int64)
    odeg = np.diff(starts)
    src_order = np.argsort(-odeg, kind="stable")
    for sid in src_order:
        ds = dst_by_src[starts[sid] : starts[sid + 1]]
        if len(ds) == 0:
            j = int(np.argmax(cap))
            cls[sid] = j
            cap[j] -= 1
            continue
        Bd = B[ds]
        rowmean = Bd.mean(1, keepdims=True)
        sc = np.maximum(Bd + 1.0 - rowmean, 0.0).sum(0) + 0.02 * Bd.sum(0)
        sc = sc + np.where(cap > 0, 0.0, 1e18)
        j = int(np.argmin(sc))
        cls[sid] = j
        cap[j] -= 1
        np.add.at(B, (ds, j), 1)
    return cls, B


# ---------------------------- host prep ------------------------------------
def _prep(lap_rows, lap_cols, lap_vals, features, W1, b1, W2, b2):
    lap_rows = np.ascontiguousarray(np.asarray(lap_rows))
    lap_cols = np.ascontiguousarray(np.asarray(lap_cols))
    lap_vals = np.ascontiguousarray(np.asarray(lap_vals), dtype=np.float32)
    features = np.ascontiguousarray(np.asarray(features), dtype=np.float32)

    cls, B = _balance_classes(lap_rows, lap_cols)

    # source relabeling: class r gets table rows r, r+4, ...
    newid = np.empty(N_NODES, np.int64)
    for r in range(NCLS):
        m = np.where(cls == r)[0]
        newid[m] = r + 4 * np.arange(len(m))
    featP = np.zeros((TAB_ROWS * 4, D), np.float32)
    featP[newid] = features
    tab = featP.reshape(TAB_ROWS, 4 * D)

    # dest striping: group by (max class count desc, argmax, B-vector)
    mx = B.max(1)
    am = B.argmax(1)
    sorder = np.lexsort((B[:, 2], B[:, 1], B[:, 0], am, -mx))
    grank = np.empty(N_NODES, np.int64)
    grank[sorder] = np.arange(N_NODES)
    # rank -> core = rank%8, pos = rank//8, tile = pos//128, part = pos%128

    # global budgets K[t, r] = max over the tile's 1024 dests (SPMD-uniform)
    K = np.zeros((T_ROWS, NCLS), np.int64)
    for t in range(T_ROWS):
        lo, hi = t * 1024, min((t + 1) * 1024, N_NODES)
        K[t] = B[sorder[lo:hi]].max(0)
    K = np.maximum(K, 1)

    # supertile grouping and global column layout
    tile_cols = K.sum(1)
    supers = []  # list of (t_lo, t_hi)
    t0 = 0
    acc = 0
    for t in range(T_ROWS):
        if acc and acc + tile_cols[t] > CMAX:
            supers.append((t0, t))
            t0, acc = t, 0
        acc += tile_cols[t]
    supers.append((t0, T_ROWS))

    colbase = np.zeros((T_ROWS, NCLS), np.int64)
    sinfo = []  # per super: (col_lo, col_hi, [(r, col_lo_r, col_hi_r)])
    col = 0
    for (ta, tb) in supers:
        s_lo = col
        blocks = []
        for r in range(NCLS):
            r_lo = col
            for t in range(ta, tb):
                colbase[t, r] = col
                col += K[t, r]
            blocks.append((r, r_lo, col))
        sinfo.append((s_lo, col, blocks))
    totcols = col

    # per-edge placement
    e_rank = grank[lap_rows]
    e_core = e_rank % CORES
    e_pos = e_rank // CORES
    e_t = e_pos // 128
    e_p = e_pos % 128
    e_cls = (newid[lap_cols] % 4).astype(np.int64)
    e_i16 = (newid[lap_cols] // 4).astype(np.int64)

    bias = (np.asarray(b1, np.float32) + np.asarray(b2, np.float32)).reshape(D, 1)
    W1 = np.ascontiguousarray(np.asarray(W1), np.float32)
    W2 = np.ascontiguousarray(np.asarray(W2), np.float32)
    ident = np.eye(128, dtype=np.float32)

    in_maps = []
    perms = []
    for c in range(CORES):
        sel = np.where(e_core == c)[0]
        t_ = e_t[sel]
        p_ = e_p[sel]
        r_ = e_cls[sel]
        i16_ = e_i16[sel]
        v_ = lap_vals[sel]
        # rank within (t, r, p) groups
        key = (t_ * NCLS + r_) * 128 + p_
        o2 = np.argsort(key, kind="stable")
        ks = key[o2]
        grp_starts = np.searchsorted(ks, np.arange(T_ROWS * NCLS * 128))
        rank_in = np.arange(len(ks)) - grp_starts[ks]
        colidx = colbase[t_[o2], r_[o2]] + rank_in
        if np.any(rank_in >= K[t_[o2], r_[o2]]):
            raise AssertionError("slot overflow")
        gidx = np.zeros((128, totcols), np.int16)
        gval = np.zeros((128, totcols), np.float32)
        gidx[p_[o2], colidx] = i16_[o2]
        gval[p_[o2], colidx] = v_[o2]

        # idx buffer in dma_gather layout: flat i = (col-a)*128 + p within
        # each per-(super, class) instruction block; [16, 8C] replicated x8
        idxbuf = np.empty((128, 8 * totcols), np.int16)
        for (s_lo, s_hi, blocks) in sinfo:
            for (r, r_lo, r_hi) in blocks:
                C = r_hi - r_lo
                if C == 0:
                    continue
                arr = gidx[:, r_lo:r_hi].T.ravel()  # i = c*128+p
                r16 = arr.reshape(C * 8, 16).T      # [16, 8C]
                idxbuf[:, 8 * r_lo : 8 * r_hi] = np.tile(r16, (8, 1))

        perm = sorder[c::CORES]  # position i -> dest id
        fT = np.zeros((D, NDP), np.float32)
        fT[:, :ND] = features[perm].T
        perms.append(perm)

        in_maps.append(
            {
                "tab": tab,
                "idx": idxbuf,
                "val": gval,
                "fT": fT,
                "W1": W1,
                "W2": W2,
                "bias": bias,
                "ident": ident,
            }
        )
    meta = {
        "K": K.tolist(),
        "supers": supers,
        "sinfo": sinfo,
        "colbase": colbase.tolist(),
        "totcols": totcols,
    }
    return in_maps, perms, meta


# --------------------------- device kernel ---------------------------------
def build_kernel(meta, gbufs=4):
    K = meta["K"]
    sinfo = meta["sinfo"]
    supers = meta["supers"]
    colbase = meta["colbase"]
    totcols = meta["totcols"]

    nc = bacc.Bacc("TRN2", target_bir_lowering=False, debug=False)

    tab = nc.dram_tensor("tab", [TAB_ROWS, 4 * D], FP32, kind="ExternalInput")
    idx_t = nc.dram_tensor("idx", [128, 8 * totcols], INT16, kind="ExternalInput")
    val_t = nc.dram_tensor("val", [128, totcols], FP32, kind="ExternalInput")
    fT = nc.dram_tensor("fT", [D, NDP], FP32, kind="ExternalInput")
    W1 = nc.dram_tensor("W1", [D, D], FP32, kind="ExternalInput")
    W2 = nc.dram_tensor("W2", [D, D], FP32, kind="ExternalInput")
    bias = nc.dram_tensor("bias", [D, 1], FP32, kind="ExternalInput")
    ident = nc.dram_tensor("ident", [128, 128], FP32, kind="ExternalInput")
    outT = nc.dram_tensor("outT", [D, NDP], FP32, kind="ExternalOutput")

    with tile.TileContext(nc) as tc:
        nc.gpsimd.load_library(library_config.mlp)
        with (
            tc.tile_pool(name="dense", bufs=1) as dpool,
            tc.tile_pool(name="gbuf", bufs=gbufs) as gpool,
            tc.tile_pool(name="xbuf", bufs=3) as xpool,
            tc.tile_pool(name="fbuf", bufs=3) as fpool,
            tc.tile_pool(name="obuf", bufs=3) as opool,
            tc.tile_pool(name="work", bufs=3) as wpool,
            tc.tile_pool(name="psum", bufs=4, space="PSUM") as pspool,
        ):
            # split the preloads so the first supertile's gathers start early
            c1 = sinfo[0][1]
            idx_all = dpool.tile([128, 8 * totcols], INT16)
            nc.sync.dma_start(out=idx_all[:, : 8 * c1], in_=idx_t[:, : 8 * c1])
            nc.sync.dma_start(out=idx_all[:, 8 * c1 :], in_=idx_t[:, 8 * c1 :])
            val_all = dpool.tile([128, totcols], FP32)
            nc.sync.dma_start(out=val_all[:, :c1], in_=val_t[:, :c1])
            nc.sync.dma_start(out=val_all[:, c1:], in_=val_t[:, c1:])
            w1_sb = dpool.tile([D, D], FP32)
            nc.sync.dma_start(out=w1_sb[:], in_=W1[:])
            w2_sb = dpool.tile([D, D], FP32)
            nc.sync.dma_start(out=w2_sb[:], in_=W2[:])
            bias_sb = dpool.tile([D, 1], FP32)
            nc.sync.dma_start(out=bias_sb[:], in_=bias[:])
            id_sb = dpool.tile([128, 128], FP32)
            nc.sync.dma_start(out=id_sb[:], in_=ident[:])

            def dense_tile(t, x_t):
                fT_t = fpool.tile([D, 128], FP32, tag="f")
                nc.sync.dma_start(out=fT_t[:], in_=fT[:, t * 128 : (t + 1) * 128])
                xT_ps = pspool.tile([D, 128], FP32, tag="xT")
                nc.tensor.transpose(out=xT_ps[:], in_=x_t[:], identity=id_sb[:])
                a_t = wpool.tile([D, 128], FP32, tag="A")
                nc.vector.tensor_tensor(
                    out=a_t[:], in0=fT_t[:], in1=xT_ps[:], op=mybir.AluOpType.add
                )
                b_t = wpool.tile([D, 128], FP32, tag="B")
                nc.vector.tensor_tensor(
                    out=b_t[:], in0=fT_t[:], in1=xT_ps[:], op=mybir.AluOpType.mult
                )
                o_ps = pspool.tile([D, 128], FP32, tag="o")
                nc.tensor.matmul(
                    o_ps[:], lhsT=w1_sb[:], rhs=a_t[:], start=True, stop=False
                )
                nc.tensor.matmul(
                    o_ps[:], lhsT=w2_sb[:], rhs=b_t[:], start=False, stop=True
                )
                ot = opool.tile([D, 128], FP32, tag="ot")
                nc.scalar.activation(
                    out=ot[:],
                    in_=o_ps[:],
                    func=mybir.ActivationFunctionType.Identity,
                    bias=bias_sb[:],
                )
                nc.sync.dma_start(out=outT[:, t * 128 : (t + 1) * 128], in_=ot[:])

            for si, (ta, tb) in enumerate(supers):
                s_lo, s_hi, blocks = sinfo[si]
                C_s = s_hi - s_lo
                G = gpool.tile([128, C_s * D], FP32, tag="G")
                for (r, r_lo, r_hi) in blocks:
                    C_r = r_hi - r_lo
                    if C_r == 0:
                        continue
                    nc.gpsimd.dma_gather(
                        out_ap=G[:, (r_lo - s_lo) * D : (r_hi - s_lo) * D].rearrange(
                            "p (c f) -> p c f", f=D
                        ),
                        in_ap=tab[:][:, r * D : (r + 1) * D],
                        idxs_ap=idx_all[:, 8 * r_lo : 8 * r_hi],
                        num_idxs=128 * C_r,
                        num_idxs_reg=128 * C_r,
                        elem_size=D,
                        elem_step=4 * D,
                        single_packet=False,
                    )
                G3 = G[:].rearrange("p (c f) -> p c f", f=D)
                nc.vector.tensor_tensor(
                    out=G3,
                    in0=G3,
                    in1=val_all[:, s_lo:s_hi, None].to_broadcast([128, C_s, D]),
                    op=mybir.AluOpType.mult,
                )
                if tb - ta == 1:
                    # single-tile supertile: all classes' columns are
                    # contiguous; one reduce sums them all
                    x_t = xpool.tile([128, D], FP32, tag="x")
                    nc.vector.tensor_reduce(
                        out=x_t[:],
                        in_=G[:].rearrange("p (c f) -> p f c", f=D),
                        axis=mybir.AxisListType.X,
                        op=mybir.AluOpType.add,
                    )
                    dense_tile(ta, x_t)
                    continue
                for t in range(ta, tb):
                    xpart = xpool.tile([128, NCLS * D], FP32, tag="xp")
                    for r in range(NCLS):
                        a = colbase[t][r] - s_lo
                        kk = K[t][r]
                        gv = G[:].rearrange("p (c f) -> p f c", f=D)[
                            :, :, a : a + kk
                        ]
                        nc.vector.tensor_reduce(
                            out=xpart[:, r * D : (r + 1) * D],
                            in_=gv,
                            axis=mybir.AxisListType.X,
                            op=mybir.AluOpType.add,
                        )
                    x_t = xpool.tile([128, D], FP32, tag="x")
                    nc.vector.tensor_reduce(
                        out=x_t[:],
                        in_=xpart[:].rearrange("p (c f) -> p f c", f=D),
                        axis=mybir.AxisListType.X,
                        op=mybir.AluOpType.add,
                    )
                    dense_tile(t, x_t)

    nc.compile()
    return nc


# ------------------------------ entry point --------------------------------
def kernel(lap_rows, lap_cols, lap_vals, features, W1, b1, W2, b2):
    in_maps, perms, meta = _prep(
        lap_rows, lap_cols, lap_vals, features, W1, b1, W2, b2
    )
    nc = build_kernel(meta)
    res = run_bass_kernel_spmd(nc, in_maps, core_ids=list(range(CORES)))
    out = np.empty((N_NODES, D), np.float32)
    for c in range(CORES):
        out[perms[c]] = res.results[c]["outT"][:, :ND].T
    return out


if __name__ == "__main__":
    # quick self-check on a numpy-generated graph (same distribution)
    rng = np.random.default_rng(0)
    inp = {
        "lap_rows": rng.integers(0, N_NODES, N_EDGES).astype(np.int32),
        "lap_cols": rng.integers(0, N_NODES, N_EDGES).astype(np.int32),
        "lap_vals": (rng.random(N_EDGES, np.float32) * 0.1),
        "features": rng.standard_normal((N_NODES, D)).astype(np.float32),
        "W1": (rng.standard_normal((D, D)) / np.sqrt(D)).astype(np.float32),
        "b1": np.zeros(D, np.float32),
        "W2": (rng.standard_normal((D, D)) / np.sqrt(D)).astype(np.float32),
        "b2": np.zeros(D, np.float32),
    }
    got = kernel(**inp)
    msgs = inp["lap_vals"][:, None] * inp["features"][inp["lap_cols"]]
    x = np.zeros_like(inp["features"])
    np.add.at(x, inp["lap_rows"], msgs)
    want = (
        (inp["features"] + x) @ inp["W1"]
        + inp["b1"]
        + (x * inp["features"]) @ inp["W2"]
        + inp["b2"]
    )
    err = np.abs(got - want)
    rel = err.max() / np.abs(want).max()
    print(f"max abs err {err.max():.3e}  rel {rel:.3e}")
